# revision 19
# baseline (speedup 1.0000x reference)
"""Trainium2 Bass kernel: image-captioning LSTM decoder (DecoderWithRNN).

Strategy (8 NeuronCores, SPMD, one NEFF, one collective):
  - init Linear (512 x 401408, 822MB of weights) is contraction-sharded over
    encoder_dim: each core streams 1/8 of init_w^T from HBM and accumulates a
    partial x0 [32,512]; a single small AllReduce produces the full x0.
  - BatchNorm (batch stats), the 51-step LSTM recurrence, and the x-side gate
    precompute are replicated on every core (the recurrence is PE-stream-bound,
    so batch replication costs nothing extra).
  - fc (vocab projection, V=10000) is sharded over V: each core emits
    preds[:, :, slice of 1250] and the host concatenates.
  - sorting / embedding lookup / weight transposes are host-side input prep.

Layouts on device (per core):
  - h state kept transposed: H2T [128part, 4 H-chunks, 53 slots * 32 batch]
    (slot 0 = h after the init cell; slot t+1 = h2 of step t; slot 52 = pad).
  - gates computed as psum [32, 512] per gate, order [g, i, f, o];
    W_hh^T / W_ih^T columns pre-permuted accordingly on host.
  - x-side gate contributions for all steps precomputed in one batched matmul
    ([51*32, 512] @ [512, 2048]), spilled to HBM, streamed back per 4 steps,
    injected into psum via an identity matmul (no DVE add on the hot path).
"""

import numpy as np

import concourse.bass as bass
import concourse.tile as tile
from concourse import mybir
from concourse.bass_utils import run_bass_kernel_spmd

B, L, V, H, E = 32, 52, 10000, 512, 512
ENC = 14 * 14 * 2048
NCORES = 8
T = L - 1  # 51 decode steps
NSLOT = 53  # h-state slots: 0 = init-cell h, 1..51 = step h2, 52 = zero pad
ENC_S = ENC // NCORES  # 50176
KT_INIT = ENC_S // 128  # 392 contraction tiles for the init matmul
ENC_CHUNK = 8  # enc^T k-tiles per DMA
VS = V // NCORES  # 1250 vocab columns per core
NM = (NSLOT * B) // 128  # 13 fc row-tiles (52 slots + pad, 32 batch)
BN_EPS = 1e-5
F32 = mybir.dt.float32
BF16 = mybir.dt.bfloat16

# gate order on device: [g, i, f, o] (tanh gate first so DVE work starts early)
GATE_PERM = np.concatenate(
    [np.arange(2 * H, 3 * H), np.arange(0, H), np.arange(H, 2 * H), np.arange(3 * H, 4 * H)]
)
IX_G, IX_I, IX_F, IX_O = 0, 1, 2, 3

FC_CHUNKS = [(0, 512), (512, 1024), (1024, VS)]


def _split_multi_waits(nc, max_waits=1):
    """walrus in this container allows a single sync-wait per instruction;
    split Tile's multi-wait tail drain into preceding single-wait NOPs."""
    n = 0
    for bb in nc.main_func.blocks:
        insts = bb.instructions
        i = 0
        while i < len(insts):
            inst = insts[i]
            si = getattr(inst, "sync_info", None)
            if si is not None and si.on_wait and len(si.on_wait) > max_waits:
                extra = si.on_wait[: -max_waits]
                si.on_wait = si.on_wait[-max_waits:]
                for k, w in enumerate(extra):
                    nop = mybir.InstNoOp(
                        name=f"{inst.name}-wsplit{k}",
                        sync_info=mybir.SyncInfo(on_wait=[w], on_update=[]),
                        engine=inst.engine,
                        bass_nofuse=True,
                    )
                    insts.insert(i, nop)
                    i += 1
                n += len(extra)
            i += 1
    return n


def _build():
    nc = bass.Bass("TRN2", target_bir_lowering=False, debug=False, num_devices=NCORES)

    # ---- I/O ----
    d_encT = nc.dram_tensor("encT", [ENC_S, B], F32, kind="ExternalInput")
    d_initwT = nc.dram_tensor("initwT", [ENC_S, H], F32, kind="ExternalInput")
    d_xT = nc.dram_tensor("xT", [H, T * B], BF16, kind="ExternalInput")
    d_wihT = nc.dram_tensor("wihT", [H, 4 * H], BF16, kind="ExternalInput")
    d_whhT = nc.dram_tensor("whhT", [H, 4 * H], F32, kind="ExternalInput")
    d_bias = nc.dram_tensor("bias", [128, 4 * H], F32, kind="ExternalInput")
    d_gb = nc.dram_tensor("gb", [128, 8], F32, kind="ExternalInput")
    d_fcwT = nc.dram_tensor("fcwT", [H, VS], F32, kind="ExternalInput")
    d_fcb = nc.dram_tensor("fcb", [1, VS], F32, kind="ExternalInput")
    d_mask = nc.dram_tensor("mask", [128, NM], F32, kind="ExternalInput")
    d_eye = nc.dram_tensor("eye", [128, 32], F32, kind="ExternalInput")
    d_preds = nc.dram_tensor("preds", [NM, 128, VS], F32, kind="ExternalOutput")

    with tile.TileContext(nc) as tc:
        with (
            tc.tile_pool(name="const", bufs=1) as constp,
            tc.tile_pool(name="encs", bufs=2) as encp,
            tc.tile_pool(name="winit", bufs=4) as winitp,
            tc.tile_pool(name="bigstage", bufs=2) as bigp,
            tc.tile_pool(name="xgwin", bufs=3) as xgwinp,
            tc.tile_pool(name="work", bufs=2) as workp,
            tc.tile_pool(name="workbn", bufs=1) as workbnp,
            tc.tile_pool(name="dram", bufs=1, space="DRAM") as dramp,
            tc.tile_pool(name="ps", bufs=6, space="PSUM") as psp,
            tc.tile_pool(name="ps_small", bufs=2, space="PSUM") as pssp,
        ):
            d_xg = dramp.tile([NM, 128, 4 * H], F32, tag="xg_spill")
            d_ccin = dramp.tile([B, H], F32, tag="cc_in")
            d_ccout = dramp.tile([B, H], F32, tag="cc_out")
            # ---- persistent constants ----
            t_wih = constp.tile([128, 4, 4 * H], BF16, tag="wih")
            t_whh = constp.tile([128, 4, 4 * H], F32, tag="whh")
            t_xT = constp.tile([128, 4, T * B], BF16, tag="xT")
            for k in range(4):
                sl = slice(128 * k, 128 * (k + 1))
                nc.sync.dma_start(out=t_wih[:, k, :], in_=d_wihT[sl, :])
                nc.sync.dma_start(out=t_whh[:, k, :], in_=d_whhT[sl, :])
                nc.sync.dma_start(out=t_xT[:, k, :], in_=d_xT[sl, :])
            t_bias = constp.tile([128, 4 * H], F32, tag="bias")
            t_gb = constp.tile([128, 8], F32, tag="gb")
            t_eye = constp.tile([128, 32], F32, tag="eye")
            nc.sync.dma_start(out=t_bias[:], in_=d_bias[:])
            nc.sync.dma_start(out=t_gb[:], in_=d_gb[:])
            nc.sync.dma_start(out=t_eye[:], in_=d_eye[:])
            t_ones = constp.tile([1, 128], F32, tag="ones")
            nc.vector.memset(t_ones[:], 1.0)
            t_eps = constp.tile([128, 1], F32, tag="eps")
            nc.vector.memset(t_eps[:], BN_EPS)

            # persistent state
            t_h2t = constp.tile([128, 4, NSLOT * B], F32, tag="h2t")
            nc.vector.memset(t_h2t[:, :, 52 * B : 53 * B], 0.0)  # pad slot
            t_c = constp.tile([B, H], F32, tag="c")

            # ---- phase A: init matmul (DMA-bound) ----
            ps_x0 = psp.tile([B, H], F32, tag="ps", name="ps_x0")
            enc_view = d_encT.ap().rearrange("(n p) b -> n p b", p=128)
            for c in range(KT_INIT // ENC_CHUNK):
                t_enc = encp.tile([128, ENC_CHUNK, B], F32, tag="enc")
                nc.sync.dma_start(
                    out=t_enc[:],
                    in_=enc_view[c * ENC_CHUNK : (c + 1) * ENC_CHUNK, :, :].rearrange(
                        "n p b -> p n b"
                    ),
                )
                for j in range(ENC_CHUNK):
                    k = c * ENC_CHUNK + j
                    t_w = winitp.tile([128, H], F32, tag="winit")
                    nc.sync.dma_start(out=t_w[:], in_=d_initwT[128 * k : 128 * (k + 1), :])
                    nc.tensor.matmul(
                        ps_x0[:],
                        lhsT=t_enc[:, j, :],
                        rhs=t_w[:],
                        start=(k == 0),
                        stop=(k == KT_INIT - 1),
                    )

            # ---- phase A': x-side gate precompute, spilled to HBM ----
            # columns are (t*32+b); tile m covers steps [4m, 4m+4); tile 12 has
            # only 96 valid rows (xT ends at t=50); its last 32 rows are unused.
            for m in range(NM):
                ps_g = [psp.tile([128, 512], F32, tag="ps", name=f"xg{m}_{g}") for g in range(4)]
                c0 = 128 * m
                ncols = 128 if m < NM - 1 else (T * B - c0)
                for g in range(4):
                    for k in range(4):
                        nc.tensor.matmul(
                            ps_g[g][:ncols, :],
                            lhsT=t_xT[:, k, c0 : c0 + ncols],
                            rhs=t_wih[:, k, 512 * g : 512 * (g + 1)],
                            start=(k == 0),
                            stop=(k == 3),
                        )
                t_stage = xgwinp.tile([128, 4 * H], F32, tag="xgwin")
                for g in range(4):
                    nc.vector.scalar_tensor_tensor(
                        out=t_stage[:ncols, 512 * g : 512 * (g + 1)],
                        in0=ps_g[g][:ncols, :],
                        scalar=1.0,
                        in1=t_bias[:ncols, 512 * g : 512 * (g + 1)],
                        op0=mybir.AluOpType.mult,
                        op1=mybir.AluOpType.add,
                    )
                nc.sync.dma_start(out=d_xg[m, :ncols, :], in_=t_stage[:ncols, :])

            # ---- phase B: AllReduce partial x0, BN, init cell ----
            t_x0 = workbnp.tile([B, H], F32, tag="x0")
            nc.vector.tensor_copy(t_x0[:], ps_x0[:])
            nc.sync.dma_start(out=d_ccin[:], in_=t_x0[:])
            nc.gpsimd.collective_compute(
                "AllReduce",
                mybir.AluOpType.add,
                ins=[d_ccin.opt()],
                outs=[d_ccout.opt()],
                replica_groups=[list(range(NCORES))],
            )
            t_x0r = workbnp.tile([B, H], F32, tag="x0r")
            nc.sync.dma_start(out=t_x0r[:], in_=d_ccout[:])

            # transpose x0 -> [128, 4, 32]
            ps_tp0 = pssp.tile([128, 128], F32, tag="ps_small", name="tp0")
            for u in range(4):
                nc.tensor.transpose(
                    ps_tp0[:, 32 * u : 32 * (u + 1)],
                    t_x0r[:, 128 * u : 128 * (u + 1)],
                    t_eye[0:32, :],
                )
            t_x0T = workbnp.tile([128, 4, B], F32, tag="x0T")
            nc.vector.tensor_copy(t_x0T[:], ps_tp0[:].rearrange("p (u b) -> p u b", b=B))

            # BatchNorm in transposed layout (stats along free dim = batch)
            t_xn = workbnp.tile([128, 4, B], BF16, tag="xn")
            for u in range(4):
                t_stats = workbnp.tile([128, 6], F32, tag="bnstats")
                nc.vector.bn_stats(out=t_stats[:], in_=t_x0T[:, u, :])
                t_mv = workbnp.tile([128, 2], F32, tag="bnmv")
                nc.vector.bn_aggr(out=t_mv[:], in_=t_stats[:])
                t_sd = workbnp.tile([128, 1], F32, tag="bnsd")
                nc.scalar.activation(
                    out=t_sd[:],
                    in_=t_mv[:, 1:2],
                    func=mybir.ActivationFunctionType.Sqrt,
                    bias=t_eps[:],
                )
                t_rstd = workbnp.tile([128, 1], F32, tag="bnrstd")
                nc.vector.reciprocal(t_rstd[:], t_sd[:])
                t_scale = workbnp.tile([128, 1], F32, tag="bnscale")
                nc.vector.tensor_mul(t_scale[:], t_rstd[:], t_gb[:, 2 * u : 2 * u + 1])
                nc.vector.tensor_scalar(
                    out=t_xn[:, u, :],
                    in0=t_x0T[:, u, :],
                    scalar1=t_mv[:, 0:1],
                    scalar2=t_scale[:],
                    op0=mybir.AluOpType.subtract,
                    op1=mybir.AluOpType.mult,
                )
                nc.vector.tensor_scalar_add(t_xn[:, u, :], t_xn[:, u, :], t_gb[:, 2 * u + 1 : 2 * u + 2])

            def cell(t, gate_rhs_fn, lhsT_fn, has_f, tau=0):
                """one LSTM cell; writes h2 transposed into slot t+1 (slot 0 for init)."""
                ps_gates = []
                for g in range(4):
                    pg = psp.tile([B, 512], F32, tag="ps", name=f"g{t}_{g}")
                    gsl = slice(512 * g, 512 * (g + 1))
                    nc.tensor.matmul(
                        pg[:],
                        lhsT=t_eye[0:32, :],
                        rhs=gate_rhs_fn(g),
                        start=True,
                        stop=False,
                    )
                    for k in range(4):
                        nc.tensor.matmul(
                            pg[:],
                            lhsT=lhsT_fn(k),
                            rhs=(t_wih if not has_f else t_whh)[:, k, gsl],
                            start=False,
                            stop=(k == 3),
                        )
                    ps_gates.append(pg)
                t_tg = workp.tile([B, H], F32, tag="tg")
                nc.scalar.activation(t_tg[:], ps_gates[IX_G][:], mybir.ActivationFunctionType.Tanh)
                t_is = workp.tile([B, H], F32, tag="is")
                nc.scalar.activation(t_is[:], ps_gates[IX_I][:], mybir.ActivationFunctionType.Sigmoid)
                t_t0 = workp.tile([B, H], F32, tag="t0")
                nc.vector.tensor_mul(t_t0[:], t_is[:], t_tg[:])
                if has_f:
                    t_fs = workp.tile([B, H], F32, tag="fs")
                    nc.scalar.activation(
                        t_fs[:], ps_gates[IX_F][:], mybir.ActivationFunctionType.Sigmoid
                    )
                    t_c1 = workp.tile([B, H], F32, tag="c1")
                    nc.vector.tensor_mul(t_c1[:], t_fs[:], t_c[:])
                    nc.vector.tensor_add(t_c[:], t_c1[:], t_t0[:])
                else:
                    nc.vector.tensor_copy(t_c[:], t_t0[:])
                t_os = workp.tile([B, H], F32, tag="os")
                nc.scalar.activation(t_os[:], ps_gates[IX_O][:], mybir.ActivationFunctionType.Sigmoid)
                t_tc = workp.tile([B, H], F32, tag="tc")
                nc.scalar.activation(t_tc[:], t_c[:], mybir.ActivationFunctionType.Tanh)
                t_h2 = workp.tile([B, H], F32, tag="h2")
                nc.vector.tensor_mul(t_h2[:], t_os[:], t_tc[:])
                # transpose h2 into slot t+1
                slot = t + 1
                ps_tp = pssp.tile([128, 128], F32, tag="ps_small", name=f"tp{t}")
                for u in range(4):
                    nc.tensor.transpose(
                        ps_tp[:, 32 * u : 32 * (u + 1)],
                        t_h2[:, 128 * u : 128 * (u + 1)],
                        t_eye[0:32, :],
                    )
                nc.vector.tensor_copy(
                    t_h2t[:, :, B * slot : B * (slot + 1)],
                    ps_tp[:].rearrange("p (u b) -> p u b", b=B),
                )

            # init cell: x = BN output, h=c=0 (skip f-gate term), bias via inject
            cell(
                -1,
                gate_rhs_fn=lambda g: t_bias[0:B, 512 * g : 512 * (g + 1)],
                lhsT_fn=lambda k: t_xn[:, k, :],
                has_f=False,
            )

            # ---- phase C: recurrence ----
            for t in range(T):
                m, tau = divmod(t, 4)
                win = xgwinp.tile([B, 4 * H], F32, tag="xgwin")
                nc.sync.dma_start(out=win[:], in_=d_xg[m, 32 * tau : 32 * (tau + 1), :])
                cell(
                    t,
                    gate_rhs_fn=lambda g, win=win: win[:, 512 * g : 512 * (g + 1)],
                    lhsT_fn=lambda k, t=t: t_h2t[:, k, B * t : B * (t + 1)],
                    has_f=True,
                )

            # ---- phase D: fc over vocab slice, bias injected, ragged mask ----
            t_fcw = constp.tile([128, 4, VS], F32, tag="xT")  # reuse released xT slot
            for k in range(4):
                nc.sync.dma_start(out=t_fcw[:, k, :], in_=d_fcwT[128 * k : 128 * (k + 1), :])
            t_fcb = constp.tile([1, VS], F32, tag="fcb")
            t_mask = constp.tile([128, NM], F32, tag="mask")
            nc.sync.dma_start(out=t_fcb[:], in_=d_fcb[:])
            nc.sync.dma_start(out=t_mask[:], in_=d_mask[:])
            for m in range(NM):
                csl = slice(B + 128 * m, B + 128 * (m + 1))  # slots 1.. (skip slot 0)
                ps_c = []
                for a, bnd in FC_CHUNKS:
                    pc = psp.tile([128, 512], F32, tag="ps", name=f"fc{m}_{a}")
                    nc.tensor.matmul(
                        pc[:, : bnd - a],
                        lhsT=t_ones[:],
                        rhs=t_fcb[:, a:bnd],
                        start=True,
                        stop=False,
                    )
                    for k in range(4):
                        nc.tensor.matmul(
                            pc[:, : bnd - a],
                            lhsT=t_h2t[:, k, csl],
                            rhs=t_fcw[:, k, a:bnd],
                            start=False,
                            stop=(k == 3),
                        )
                    ps_c.append(pc)
                t_out = bigp.tile([128, VS], F32, tag="bigstage")
                for (a, bnd), pc in zip(FC_CHUNKS, ps_c):
                    nc.vector.tensor_scalar_mul(
                        t_out[:, a:bnd], pc[:, : bnd - a], t_mask[:, m : m + 1]
                    )
                nc.sync.dma_start(out=d_preds[m, :, :], in_=t_out[:])

    _split_multi_waits(nc)
    return nc


_NC_CACHE = None


def _get_nc():
    global _NC_CACHE
    if _NC_CACHE is None:
        _NC_CACHE = _build()
    return _NC_CACHE


def kernel(
    encoder_out,
    encoded_captions,
    caption_lengths,
    emb_w,
    init_w,
    init_b,
    bn_gamma,
    bn_beta,
    w_ih,
    w_hh,
    b_ih,
    b_hh,
    fc_w,
    fc_b,
    **_unused,
):
    encoder_out = np.asarray(encoder_out, dtype=np.float32)
    encoded_captions = np.asarray(encoded_captions)
    caption_lengths = np.asarray(caption_lengths)
    emb_w = np.asarray(emb_w, dtype=np.float32)
    init_w = np.asarray(init_w, dtype=np.float32)
    init_b = np.asarray(init_b, dtype=np.float32)
    bn_gamma = np.asarray(bn_gamma, dtype=np.float32)
    bn_beta = np.asarray(bn_beta, dtype=np.float32)
    w_ih = np.asarray(w_ih, dtype=np.float32)
    w_hh = np.asarray(w_hh, dtype=np.float32)
    b_ih = np.asarray(b_ih, dtype=np.float32)
    b_hh = np.asarray(b_hh, dtype=np.float32)
    fc_w = np.asarray(fc_w, dtype=np.float32)
    fc_b = np.asarray(fc_b, dtype=np.float32)

    # ---- host-side prep (index ops, sorting, transposes) ----
    lengths = caption_lengths[:, 0]
    sort_ind = np.argsort(-lengths, kind="stable")
    lengths_s = lengths[sort_ind]
    caps = encoded_captions[sort_ind]
    dec_len = (lengths_s - 1).astype(lengths.dtype)
    enc = encoder_out.reshape(B, -1)[sort_ind]  # [B, ENC]

    emb = emb_w[caps[:, :T]]  # [B, T, E]
    xT = np.ascontiguousarray(emb.transpose(2, 1, 0).reshape(E, T * B))  # col = t*32+b

    wihT = np.ascontiguousarray(w_ih[GATE_PERM].T)  # [H, 4H]
    whhT = np.ascontiguousarray(w_hh[GATE_PERM].T)
    bias = (b_ih + b_hh)[GATE_PERM]
    bias_tile = np.tile(bias, (128, 1)).astype(np.float32)

    gb = np.empty((128, 8), np.float32)
    for u in range(4):
        gb[:, 2 * u] = bn_gamma[128 * u : 128 * (u + 1)]
        gb[:, 2 * u + 1] = bn_beta[128 * u : 128 * (u + 1)]

    # mask[r, m]: row r=32*tau+b of fc tile m covers step t=4m+tau
    mask = np.zeros((128, NM), np.float32)
    for m in range(NM):
        for tau in range(4):
            t = 4 * m + tau
            if t < T:
                mask[32 * tau : 32 * (tau + 1), m] = (dec_len > t).astype(np.float32)

    initwT_full = np.ascontiguousarray(init_w.T)  # [ENC, H]
    eye = np.tile(np.eye(32, dtype=np.float32), (4, 1))  # [128, 32]

    in_maps = []
    for c in range(NCORES):
        sl = slice(c * ENC_S, (c + 1) * ENC_S)
        vsl = slice(c * VS, (c + 1) * VS)
        in_maps.append(
            {
                "encT": np.ascontiguousarray(enc[:, sl].T),
                "initwT": np.ascontiguousarray(initwT_full[sl]),
                "xT": _to_bf16(xT),
                "wihT": _to_bf16(wihT),
                "whhT": whhT,
                "bias": bias_tile,
                "gb": gb,
                "fcwT": np.ascontiguousarray(fc_w[vsl].T),
                "fcb": fc_b[vsl].reshape(1, VS).astype(np.float32),
                "mask": mask,
                "eye": eye,
            }
        )

    nc = _get_nc()
    res = run_bass_kernel_spmd(nc, in_maps, core_ids=list(range(NCORES)))

    # ---- assemble ----
    predictions = np.empty((B, T, V), np.float32)
    for c, r in enumerate(res.results):
        arr = r["preds"].reshape(NM, 4, B, VS)  # [m, tau, b, v]
        part = arr.transpose(2, 0, 1, 3).reshape(B, NM * 4, VS)[:, :T]
        predictions[:, :, c * VS : (c + 1) * VS] = part

    sort_ind_out = sort_ind.astype(np.int32)
    return predictions, caps, dec_len, sort_ind_out


def _to_bf16(a):
    import ml_dtypes

    return a.astype(ml_dtypes.bfloat16)


# revision 30
# speedup vs baseline: 1.9048x; 1.9048x over previous
"""Trainium2 Bass kernel: image-captioning LSTM decoder (DecoderWithRNN).

Strategy (8 NeuronCores, SPMD, one NEFF, one collective):
  - init Linear (512 x 401408, 822MB of weights) is contraction-sharded over
    encoder_dim: each core streams 1/8 of init_w^T (bf16) from HBM and
    accumulates a partial x0 [32,512]; one small AllReduce produces full x0.
  - BatchNorm (batch stats), the 51-step LSTM recurrence, and the x-side gate
    precompute are replicated on every core (the recurrence is PE-bound;
    batch replication costs nothing extra).
  - fc (vocab projection, V=10000) is sharded over V: each core emits
    preds[:, :, slice of 1250] and the host concatenates.
  - sorting / embedding lookup / weight transposes are host-side input prep.

Recurrence layout ("orientation B"): gates are computed TRANSPOSED —
  gatesT[j, b] = sum_k W'[j, k] h[k, b], with W' row-permuted to [g,i,f,o].
  j lives on partitions (16 tiles of 128), batch on the free dim. All 64
  accumulating matmuls of a step target ONE psum bank [128, (4g,4u,32b)].
  The elementwise chain then runs at full 128-partition occupancy and h2 is
  produced directly in the transposed h-state layout H2T [128, u, slot*32+b]
  (no transposes on the hot path). The x-side contributions (x_t @ W_ih^T +
  biases) for all steps are precomputed in one batched matmul, spilled to HBM
  transposed, and streamed back one [128, 512] bf16 window per step.
"""

import numpy as np

import concourse.bass as bass
import concourse.tile as tile
from concourse import mybir
from concourse.bass_utils import run_bass_kernel_spmd

B, L, V, H, E = 32, 52, 10000, 512, 512
ENC = 14 * 14 * 2048
NCORES = 8
T = L - 1  # 51 decode steps
NSLOT = 53  # h-state slots: 0 = init-cell h, 1..51 = step h2, 52 = zero pad
ENC_S = ENC // NCORES  # 50176
KT_INIT = ENC_S // 128  # 392 contraction tiles for the init matmul
ENC_CHUNK = 8  # enc^T k-tiles per DMA
VS = V // NCORES  # 1250 vocab columns per core
NM = (NSLOT * B) // 128  # 13 fc row-tiles (52 slots + pad, 32 batch)
NJ = 16  # 128-wide j tiles of the 2048 gate dim
BN_EPS = 1e-5
F32 = mybir.dt.float32
BF16 = mybir.dt.bfloat16

# gate order on device: [g, i, f, o]
GATE_PERM = np.concatenate(
    [np.arange(2 * H, 3 * H), np.arange(0, H), np.arange(H, 2 * H), np.arange(3 * H, 4 * H)]
)

FC_CHUNKS = [(0, 512), (512, 1024), (1024, VS)]
XG_CHUNKS = [(0, 512), (512, 1024), (1024, 1536), (1536, T * B)]


def _split_multi_waits(nc, max_waits=1):
    """walrus in this container allows a single sync-wait per instruction;
    split Tile's multi-wait tail drain into preceding single-wait NOPs."""
    n = 0
    for bb in nc.main_func.blocks:
        insts = bb.instructions
        i = 0
        while i < len(insts):
            inst = insts[i]
            si = getattr(inst, "sync_info", None)
            if si is not None and si.on_wait and len(si.on_wait) > max_waits:
                extra = si.on_wait[: -max_waits]
                si.on_wait = si.on_wait[-max_waits:]
                for k, w in enumerate(extra):
                    nop = mybir.InstNoOp(
                        name=f"{inst.name}-wsplit{k}",
                        sync_info=mybir.SyncInfo(on_wait=[w], on_update=[]),
                        engine=inst.engine,
                        bass_nofuse=True,
                    )
                    insts.insert(i, nop)
                    i += 1
                n += len(extra)
            i += 1
    return n


def _build():
    nc = bass.Bass("TRN2", target_bir_lowering=False, debug=False, num_devices=NCORES)

    # ---- I/O ----
    d_encT = nc.dram_tensor("encT", [ENC_S, B], BF16, kind="ExternalInput")
    d_initwT = nc.dram_tensor("initwT", [ENC_S, H], BF16, kind="ExternalInput")
    d_xT = nc.dram_tensor("xT", [H, T * B], BF16, kind="ExternalInput")
    d_wihT = nc.dram_tensor("wihT", [H, 4 * H], BF16, kind="ExternalInput")
    d_whhT = nc.dram_tensor("whhT", [H, 4 * H], BF16, kind="ExternalInput")
    d_biasB = nc.dram_tensor("biasB", [128, NJ], F32, kind="ExternalInput")
    d_biasrow = nc.dram_tensor("biasrow", [1, 4 * H], F32, kind="ExternalInput")
    d_gb = nc.dram_tensor("gb", [128, 8], F32, kind="ExternalInput")
    d_fcwT = nc.dram_tensor("fcwT", [H, VS], BF16, kind="ExternalInput")
    d_fcb = nc.dram_tensor("fcb", [1, VS], F32, kind="ExternalInput")
    d_mask = nc.dram_tensor("mask", [128, NM], F32, kind="ExternalInput")
    d_eye = nc.dram_tensor("eye", [32, 32], F32, kind="ExternalInput")
    d_preds = nc.dram_tensor("preds", [NM, 128, VS], F32, kind="ExternalOutput")

    with tile.TileContext(nc) as tc:
        with (
            tc.tile_pool(name="const", bufs=1) as constp,
            tc.tile_pool(name="encs", bufs=2) as encp,
            tc.tile_pool(name="winit", bufs=4) as winitp,
            tc.tile_pool(name="bigstage", bufs=2) as bigp,
            tc.tile_pool(name="xgwin", bufs=4) as xgwinp,
            tc.tile_pool(name="work", bufs=2) as workp,
            tc.tile_pool(name="workbn", bufs=1) as workbnp,
            tc.tile_pool(name="dram", bufs=1, space="DRAM") as dramp,
            tc.tile_pool(name="ps", bufs=6, space="PSUM") as psp,
            tc.tile_pool(name="ps_small", bufs=2, space="PSUM") as pssp,
        ):
            # xg spill: per step a [128(j-within-tile), jb=4g+u, 32(b)] block
            d_xg = dramp.tile([T, 128, NJ, B], BF16, tag="xg_spill")
            d_ccin = dramp.tile([B, H], F32, tag="cc_in")
            d_ccout = dramp.tile([B, H], F32, tag="cc_out")

            # ---- persistent constants ----
            t_wih = constp.tile([128, 4, 4 * H], BF16, tag="wih")
            t_whh = constp.tile([128, 4, 4 * H], BF16, tag="whh")
            t_xT = constp.tile([128, 4, T * B], BF16, tag="xT")
            for k in range(4):
                sl = slice(128 * k, 128 * (k + 1))
                nc.sync.dma_start(out=t_wih[:, k, :], in_=d_wihT[sl, :])
                nc.sync.dma_start(out=t_whh[:, k, :], in_=d_whhT[sl, :])
                nc.sync.dma_start(out=t_xT[:, k, :], in_=d_xT[sl, :])
            t_biasB = constp.tile([128, NJ], F32, tag="biasB")
            t_biasrow = constp.tile([1, 4 * H], F32, tag="biasrow")
            t_gb = constp.tile([128, 8], F32, tag="gb")
            t_eye = constp.tile([32, 32], F32, tag="eye")
            nc.sync.dma_start(out=t_biasB[:], in_=d_biasB[:])
            nc.sync.dma_start(out=t_biasrow[:], in_=d_biasrow[:])
            nc.sync.dma_start(out=t_gb[:], in_=d_gb[:])
            nc.sync.dma_start(out=t_eye[:], in_=d_eye[:])
            t_ones = constp.tile([1, 128], F32, tag="ones")
            nc.vector.memset(t_ones[:], 1.0)
            t_ones32 = constp.tile([1, 32], F32, tag="ones32")
            nc.vector.memset(t_ones32[:], 1.0)
            t_eps = constp.tile([128, 1], F32, tag="eps")
            nc.vector.memset(t_eps[:], BN_EPS)

            # persistent state: h transposed, c transposed ([128, u, 32])
            t_h2t = constp.tile([128, 4, NSLOT * B], BF16, tag="h2t")
            nc.vector.memset(t_h2t[:, :, 52 * B : 53 * B], 0.0)  # pad slot
            t_cB = constp.tile([128, 4, B], F32, tag="cB")

            # ---- phase A: init matmul (DMA-bound) ----
            _sc = nc.enter_named_scope("init_mm", False)[0]
            ps_x0 = psp.tile([B, H], F32, tag="ps", name="ps_x0")
            enc_view = d_encT.ap().rearrange("(n p) b -> n p b", p=128)
            for c in range(KT_INIT // ENC_CHUNK):
                t_enc = encp.tile([128, ENC_CHUNK, B], BF16, tag="enc")
                nc.sync.dma_start(
                    out=t_enc[:],
                    in_=enc_view[c * ENC_CHUNK : (c + 1) * ENC_CHUNK, :, :].rearrange(
                        "n p b -> p n b"
                    ),
                )
                for j in range(ENC_CHUNK):
                    k = c * ENC_CHUNK + j
                    t_w = winitp.tile([128, H], BF16, tag="winit")
                    nc.sync.dma_start(out=t_w[:], in_=d_initwT[128 * k : 128 * (k + 1), :])
                    nc.tensor.matmul(
                        ps_x0[:],
                        lhsT=t_enc[:, j, :],
                        rhs=t_w[:],
                        start=(k == 0),
                        stop=(k == KT_INIT - 1),
                    )
            nc.leave_named_scope("init_mm", _sc, False)

            # ---- phase A': x-side gates, computed transposed, spilled to HBM ----
            # xgT[j, (t,b)] = sum_e W_ih'[j, e] x[(t,b), e] + bias'[j]
            _sc = nc.enter_named_scope("xgates", False)[0]
            for jb in range(NJ):
                jsl = slice(128 * jb, 128 * (jb + 1))
                for a, bnd in XG_CHUNKS:
                    pg = psp.tile([128, 512], F32, tag="ps", name=f"xg{jb}_{a}")
                    for k in range(4):
                        nc.tensor.matmul(
                            pg[:, : bnd - a],
                            lhsT=t_wih[:, k, jsl],
                            rhs=t_xT[:, k, a:bnd],
                            start=(k == 0),
                            stop=(k == 3),
                        )
                    t_stage = xgwinp.tile([128, 512], BF16, tag="xgstage")
                    nc.vector.tensor_scalar_add(
                        t_stage[:, : bnd - a], pg[:, : bnd - a], t_biasB[:, jb : jb + 1]
                    )
                    nsteps = (bnd - a) // B
                    t0s = a // B
                    nc.sync.dma_start(
                        out=d_xg[t0s : t0s + nsteps, :, jb, :].rearrange("t p b -> p t b"),
                        in_=t_stage[:, : bnd - a].rearrange("p (t b) -> p t b", b=B),
                    )
            nc.leave_named_scope("xgates", _sc, False)

            # ---- phase B: AllReduce partial x0, BN ----
            _sc = nc.enter_named_scope("ar_bn", False)[0]
            t_x0 = workbnp.tile([B, H], F32, tag="x0")
            nc.vector.tensor_copy(t_x0[:], ps_x0[:])
            nc.sync.dma_start(out=d_ccin[:], in_=t_x0[:])
            nc.gpsimd.collective_compute(
                "AllReduce",
                mybir.AluOpType.add,
                ins=[d_ccin.opt()],
                outs=[d_ccout.opt()],
                replica_groups=[list(range(NCORES))],
            )
            t_x0r = workbnp.tile([B, H], F32, tag="x0r")
            nc.sync.dma_start(out=t_x0r[:], in_=d_ccout[:])

            # transpose x0 -> [128, 4, 32]
            ps_tp0 = pssp.tile([128, 128], F32, tag="ps_small", name="tp0")
            for u in range(4):
                nc.tensor.transpose(
                    ps_tp0[:, 32 * u : 32 * (u + 1)],
                    t_x0r[:, 128 * u : 128 * (u + 1)],
                    t_eye[:],
                )
            t_x0T = workbnp.tile([128, 4, B], F32, tag="x0T")
            nc.vector.tensor_copy(t_x0T[:], ps_tp0[:].rearrange("p (u b) -> p u b", b=B))

            # BatchNorm in transposed layout (stats along free dim = batch)
            t_xn = workbnp.tile([128, 4, B], BF16, tag="xn")
            for u in range(4):
                t_stats = workbnp.tile([128, 6], F32, tag="bnstats")
                nc.vector.bn_stats(out=t_stats[:], in_=t_x0T[:, u, :])
                t_mv = workbnp.tile([128, 2], F32, tag="bnmv")
                nc.vector.bn_aggr(out=t_mv[:], in_=t_stats[:])
                t_sd = workbnp.tile([128, 1], F32, tag="bnsd")
                nc.scalar.activation(
                    out=t_sd[:],
                    in_=t_mv[:, 1:2],
                    func=mybir.ActivationFunctionType.Sqrt,
                    bias=t_eps[:],
                )
                t_rstd = workbnp.tile([128, 1], F32, tag="bnrstd")
                nc.vector.reciprocal(t_rstd[:], t_sd[:])
                t_scale = workbnp.tile([128, 1], F32, tag="bnscale")
                nc.vector.tensor_mul(t_scale[:], t_rstd[:], t_gb[:, 2 * u : 2 * u + 1])
                nc.vector.tensor_scalar(
                    out=t_xn[:, u, :],
                    in0=t_x0T[:, u, :],
                    scalar1=t_mv[:, 0:1],
                    scalar2=t_scale[:],
                    op0=mybir.AluOpType.subtract,
                    op1=mybir.AluOpType.mult,
                )
                nc.vector.tensor_scalar_add(
                    t_xn[:, u, :], t_xn[:, u, :], t_gb[:, 2 * u + 1 : 2 * u + 2]
                )
            nc.leave_named_scope("ar_bn", _sc, False)

            def cell_b(t, w_tile, rhs_fn, win, has_f):
                """LSTM cell, transposed-gates orientation.

                psum [128, (4g, 4u, 32b)] accumulated by 64 matmuls; win is the
                x-side+bias tile [128, (g,u), 32] bf16 (None => inject bias via
                K=1 matmuls, used for the init cell)."""
                pg = psp.tile([128, 4, 4, B], F32, tag="ps", name=f"g{t}")
                for g in range(4):
                    for u in range(4):
                        jb = 4 * g + u
                        jsl = slice(128 * jb, 128 * (jb + 1))
                        if win is None:
                            nc.tensor.matmul(
                                pg[:, g, u, :],
                                lhsT=t_biasrow[:, jsl],
                                rhs=t_ones32[:],
                                start=True,
                                stop=False,
                            )
                        for k in range(4):
                            nc.tensor.matmul(
                                pg[:, g, u, :],
                                lhsT=w_tile[:, k, jsl],
                                rhs=rhs_fn(k),
                                start=(win is not None and k == 0),
                                stop=(k == 3),
                            )
                # gsum = psum (+ win): [128, (g,u,b)] bf16
                t_gs = workp.tile([128, 4, 4, B], BF16, tag="gs")
                if win is None:
                    nc.vector.tensor_copy(t_gs[:], pg[:])
                else:
                    nc.vector.tensor_add(t_gs[:], pg[:], win[:])
                # activations: tanh on g block, sigmoid on i,f,o blocks
                t_tg = workp.tile([128, 4, B], BF16, tag="tg")
                nc.scalar.activation(t_tg[:], t_gs[:, 0], mybir.ActivationFunctionType.Tanh)
                t_sig = workp.tile([128, 3, 4, B], BF16, tag="sig")
                nc.scalar.activation(
                    t_sig[:], t_gs[:, 1:4], mybir.ActivationFunctionType.Sigmoid
                )
                # c update (cB stays f32)
                t_t0 = workp.tile([128, 4, B], BF16, tag="t0")
                nc.vector.tensor_mul(t_t0[:], t_sig[:, 0], t_tg[:])
                if has_f:
                    t_c1 = workp.tile([128, 4, B], F32, tag="c1")
                    nc.vector.tensor_mul(t_c1[:], t_sig[:, 1], t_cB[:])
                    nc.vector.tensor_add(t_cB[:], t_c1[:], t_t0[:])
                else:
                    nc.vector.tensor_copy(t_cB[:], t_t0[:])
                t_tc = workp.tile([128, 4, B], BF16, tag="tc")
                nc.scalar.activation(t_tc[:], t_cB[:], mybir.ActivationFunctionType.Tanh)
                # h2 straight into the transposed h-state
                slot = t + 1
                nc.vector.tensor_mul(
                    t_h2t[:, :, B * slot : B * (slot + 1)], t_sig[:, 2], t_tc[:]
                )

            # init cell: x = BN output (rhs), h=c=0 (skip f), bias via K=1 inject
            _sc = nc.enter_named_scope("cell0", False)[0]
            cell_b(-1, t_wih, lambda k: t_xn[:, k, :], win=None, has_f=False)
            nc.leave_named_scope("cell0", _sc, False)

            # ---- phase C: recurrence ----
            _sc = nc.enter_named_scope("recur", False)[0]
            for t in range(T):
                win = xgwinp.tile([128, 4, 4, B], BF16, tag="xgwin")
                nc.sync.dma_start(
                    out=win[:], in_=d_xg[t, :, :, :].rearrange("p j b -> p (j b)")
                )
                cell_b(
                    t,
                    t_whh,
                    lambda k, t=t: t_h2t[:, k, B * t : B * (t + 1)],
                    win=win,
                    has_f=True,
                )
            nc.leave_named_scope("recur", _sc, False)

            # ---- phase D: fc over vocab slice, bias injected, ragged mask ----
            _sc = nc.enter_named_scope("fc", False)[0]
            t_fcw = constp.tile([128, 4, VS], BF16, tag="xT")  # reuse released xT slot
            for k in range(4):
                nc.sync.dma_start(out=t_fcw[:, k, :], in_=d_fcwT[128 * k : 128 * (k + 1), :])
            t_fcb = constp.tile([1, VS], F32, tag="fcb")
            t_mask = constp.tile([128, NM], F32, tag="mask")
            nc.sync.dma_start(out=t_fcb[:], in_=d_fcb[:])
            nc.sync.dma_start(out=t_mask[:], in_=d_mask[:])
            for m in range(NM):
                csl = slice(B + 128 * m, B + 128 * (m + 1))  # slots 1.. (skip slot 0)
                ps_c = []
                for a, bnd in FC_CHUNKS:
                    pc = psp.tile([128, 512], F32, tag="ps", name=f"fc{m}_{a}")
                    nc.tensor.matmul(
                        pc[:, : bnd - a],
                        lhsT=t_ones[:],
                        rhs=t_fcb[:, a:bnd],
                        start=True,
                        stop=False,
                    )
                    for k in range(4):
                        nc.tensor.matmul(
                            pc[:, : bnd - a],
                            lhsT=t_h2t[:, k, csl],
                            rhs=t_fcw[:, k, a:bnd],
                            start=False,
                            stop=(k == 3),
                        )
                    ps_c.append(pc)
                t_out = bigp.tile([128, VS], F32, tag="bigstage")
                for (a, bnd), pc in zip(FC_CHUNKS, ps_c):
                    nc.vector.tensor_scalar_mul(
                        t_out[:, a:bnd], pc[:, : bnd - a], t_mask[:, m : m + 1]
                    )
                nc.sync.dma_start(out=d_preds[m, :, :], in_=t_out[:])
            nc.leave_named_scope("fc", _sc, False)

    _split_multi_waits(nc)
    return nc


_NC_CACHE = None


def _get_nc():
    global _NC_CACHE
    if _NC_CACHE is None:
        _NC_CACHE = _build()
    return _NC_CACHE


def _to_bf16(a):
    """fast float32 -> bfloat16 with round-to-nearest-even (finite inputs)."""
    import ml_dtypes

    a = np.ascontiguousarray(a, dtype=np.float32)
    u = a.reshape(-1).view(np.uint32)
    r = ((u >> np.uint32(16)) & np.uint32(1)) + np.uint32(0x7FFF)
    out = ((u + r) >> np.uint32(16)).astype(np.uint16)
    return out.view(ml_dtypes.bfloat16).reshape(a.shape)


def kernel(
    encoder_out,
    encoded_captions,
    caption_lengths,
    emb_w,
    init_w,
    init_b,
    bn_gamma,
    bn_beta,
    w_ih,
    w_hh,
    b_ih,
    b_hh,
    fc_w,
    fc_b,
    **_unused,
):
    encoder_out = np.asarray(encoder_out, dtype=np.float32)
    encoded_captions = np.asarray(encoded_captions)
    caption_lengths = np.asarray(caption_lengths)
    emb_w = np.asarray(emb_w, dtype=np.float32)
    init_w = np.asarray(init_w, dtype=np.float32)
    bn_gamma = np.asarray(bn_gamma, dtype=np.float32)
    bn_beta = np.asarray(bn_beta, dtype=np.float32)
    w_ih = np.asarray(w_ih, dtype=np.float32)
    w_hh = np.asarray(w_hh, dtype=np.float32)
    b_ih = np.asarray(b_ih, dtype=np.float32)
    b_hh = np.asarray(b_hh, dtype=np.float32)
    fc_w = np.asarray(fc_w, dtype=np.float32)
    fc_b = np.asarray(fc_b, dtype=np.float32)

    # ---- host-side prep (index ops, sorting, transposes) ----
    lengths = caption_lengths[:, 0]
    sort_ind = np.argsort(-lengths, kind="stable")
    lengths_s = lengths[sort_ind]
    caps = encoded_captions[sort_ind]
    dec_len = (lengths_s - 1).astype(lengths.dtype)
    enc = encoder_out.reshape(B, -1)[sort_ind]  # [B, ENC]

    emb = emb_w[caps[:, :T]]  # [B, T, E]
    xT = np.ascontiguousarray(emb.transpose(2, 1, 0).reshape(E, T * B))  # col = t*32+b

    wihT = np.ascontiguousarray(w_ih[GATE_PERM].T)  # [H, 4H]
    whhT = np.ascontiguousarray(w_hh[GATE_PERM].T)
    bias = (b_ih + b_hh)[GATE_PERM].astype(np.float32)
    biasB = np.ascontiguousarray(bias.reshape(NJ, 128).T)  # [128, NJ]

    gb = np.empty((128, 8), np.float32)
    for u in range(4):
        gb[:, 2 * u] = bn_gamma[128 * u : 128 * (u + 1)]
        gb[:, 2 * u + 1] = bn_beta[128 * u : 128 * (u + 1)]

    # mask[r, m]: row r=32*tau+b of fc tile m covers step t=4m+tau
    mask = np.zeros((128, NM), np.float32)
    for m in range(NM):
        for tau in range(4):
            t = 4 * m + tau
            if t < T:
                mask[32 * tau : 32 * (tau + 1), m] = (dec_len > t).astype(np.float32)

    iw16 = _to_bf16(init_w)  # [H, ENC] bf16
    initwT16 = np.ascontiguousarray(iw16.T)  # [ENC, H] bf16
    enc16 = _to_bf16(enc)
    eye = np.eye(32, dtype=np.float32)
    xT16 = _to_bf16(xT)
    wihT16 = _to_bf16(wihT)
    whhT16 = _to_bf16(whhT)

    in_maps = []
    for c in range(NCORES):
        sl = slice(c * ENC_S, (c + 1) * ENC_S)
        vsl = slice(c * VS, (c + 1) * VS)
        in_maps.append(
            {
                "encT": np.ascontiguousarray(enc16[:, sl].T),
                "initwT": initwT16[sl],
                "xT": xT16,
                "wihT": wihT16,
                "whhT": whhT16,
                "biasB": biasB,
                "biasrow": bias.reshape(1, 4 * H),
                "gb": gb,
                "fcwT": _to_bf16(np.ascontiguousarray(fc_w[vsl].T)),
                "fcb": fc_b[vsl].reshape(1, VS).astype(np.float32),
                "mask": mask,
                "eye": eye,
            }
        )

    nc = _get_nc()
    res = run_bass_kernel_spmd(nc, in_maps, core_ids=list(range(NCORES)))

    # ---- assemble ----
    predictions = np.empty((B, T, V), np.float32)
    for c, r in enumerate(res.results):
        arr = r["preds"].reshape(NM, 4, B, VS)  # [m, tau, b, v]
        part = arr.transpose(2, 0, 1, 3).reshape(B, NM * 4, VS)[:, :T]
        predictions[:, :, c * VS : (c + 1) * VS] = part

    return predictions, caps, dec_len, sort_ind.astype(np.int32)


# revision 33
# speedup vs baseline: 2.1366x; 1.1217x over previous
"""Trainium2 Bass kernel: image-captioning LSTM decoder (DecoderWithRNN).

Strategy (8 NeuronCores, SPMD, one NEFF, one collective):
  - init Linear (512 x 401408, 822MB of weights) is contraction-sharded over
    encoder_dim: each core streams 1/8 of init_w^T (bf16) from HBM and
    accumulates a partial x0 [32,512]; one small AllReduce produces full x0.
  - BatchNorm (batch stats), the 51-step LSTM recurrence, and the x-side gate
    precompute are replicated on every core (the recurrence is PE-bound;
    batch replication costs nothing extra).
  - fc (vocab projection, V=10000) is sharded over V: each core emits
    preds[:, :, slice of 1250] and the host concatenates.
  - sorting / embedding lookup / weight transposes are host-side input prep.

Recurrence layout ("orientation B"): gates are computed TRANSPOSED —
  gatesT[j, b] = sum_k W'[j, k] h[k, b], with W' row-permuted to [g,i,f,o].
  j lives on partitions (16 tiles of 128), batch on the free dim. All 64
  accumulating matmuls of a step target ONE psum bank [128, (4g,4u,32b)].
  The elementwise chain then runs at full 128-partition occupancy and h2 is
  produced directly in the transposed h-state layout H2T [128, u, slot*32+b]
  (no transposes on the hot path). The x-side contributions (x_t @ W_ih^T +
  biases) for all steps are precomputed in one batched matmul, spilled to HBM
  transposed, and streamed back one [128, 512] bf16 window per step.
"""

import numpy as np

import concourse.bass as bass
import concourse.tile as tile
from concourse import mybir
from concourse.bass_utils import run_bass_kernel_spmd

B, L, V, H, E = 32, 52, 10000, 512, 512
ENC = 14 * 14 * 2048
NCORES = 8
T = L - 1  # 51 decode steps
NSLOT = 53  # h-state slots: 0 = init-cell h, 1..51 = step h2, 52 = zero pad
ENC_S = ENC // NCORES  # 50176
KT_INIT = ENC_S // 128  # 392 contraction tiles for the init matmul
ENC_CHUNK = 8  # enc^T k-tiles per DMA
VS = V // NCORES  # 1250 vocab columns per core
NM = (NSLOT * B) // 128  # 13 fc row-tiles (52 slots + pad, 32 batch)
NJ = 16  # 128-wide j tiles of the 2048 gate dim
BN_EPS = 1e-5
F32 = mybir.dt.float32
BF16 = mybir.dt.bfloat16

# gate order on device: [g, i, f, o]
GATE_PERM = np.concatenate(
    [np.arange(2 * H, 3 * H), np.arange(0, H), np.arange(H, 2 * H), np.arange(3 * H, 4 * H)]
)

FC_CHUNKS = [(0, 512), (512, 1024), (1024, VS)]
XG_CHUNKS = [(0, 512), (512, 1024), (1024, 1536), (1536, T * B)]


def _split_multi_waits(nc, max_waits=1):
    """walrus in this container allows a single sync-wait per instruction;
    split Tile's multi-wait tail drain into preceding single-wait NOPs."""
    n = 0
    for bb in nc.main_func.blocks:
        insts = bb.instructions
        i = 0
        while i < len(insts):
            inst = insts[i]
            si = getattr(inst, "sync_info", None)
            if si is not None and si.on_wait and len(si.on_wait) > max_waits:
                extra = si.on_wait[: -max_waits]
                si.on_wait = si.on_wait[-max_waits:]
                for k, w in enumerate(extra):
                    nop = mybir.InstNoOp(
                        name=f"{inst.name}-wsplit{k}",
                        sync_info=mybir.SyncInfo(on_wait=[w], on_update=[]),
                        engine=inst.engine,
                        bass_nofuse=True,
                    )
                    insts.insert(i, nop)
                    i += 1
                n += len(extra)
            i += 1
    return n


def _build():
    nc = bass.Bass("TRN2", target_bir_lowering=False, debug=False, num_devices=NCORES)

    # ---- I/O ----
    d_encT = nc.dram_tensor("encT", [ENC_S, B], BF16, kind="ExternalInput")
    d_initwT = nc.dram_tensor("initwT", [ENC_S, H], BF16, kind="ExternalInput")
    d_xT = nc.dram_tensor("xT", [H, T * B], BF16, kind="ExternalInput")
    d_wihT = nc.dram_tensor("wihT", [H, 4 * H], BF16, kind="ExternalInput")
    d_whhT = nc.dram_tensor("whhT", [H, 4 * H], BF16, kind="ExternalInput")
    d_biasB = nc.dram_tensor("biasB", [128, NJ], F32, kind="ExternalInput")
    d_biasrow = nc.dram_tensor("biasrow", [1, 4 * H], F32, kind="ExternalInput")
    d_gb = nc.dram_tensor("gb", [128, 8], F32, kind="ExternalInput")
    d_fcwT = nc.dram_tensor("fcwT", [H, VS], BF16, kind="ExternalInput")
    d_fcb = nc.dram_tensor("fcb", [1, VS], F32, kind="ExternalInput")
    d_mask = nc.dram_tensor("mask", [128, NM], F32, kind="ExternalInput")
    d_eye = nc.dram_tensor("eye", [32, 32], F32, kind="ExternalInput")
    d_preds = nc.dram_tensor("preds", [NM, 128, VS], F32, kind="ExternalOutput")

    with tile.TileContext(nc) as tc:
        with (
            tc.tile_pool(name="const", bufs=1) as constp,
            tc.tile_pool(name="encs", bufs=2) as encp,
            tc.tile_pool(name="winit", bufs=3) as winitp,
            tc.tile_pool(name="bigstage", bufs=2) as bigp,
            tc.tile_pool(name="xgwin", bufs=4) as xgwinp,
            tc.tile_pool(name="work", bufs=2) as workp,
            tc.tile_pool(name="workbn", bufs=1) as workbnp,
            tc.tile_pool(name="dram", bufs=1, space="DRAM") as dramp,
            tc.tile_pool(name="ps", bufs=6, space="PSUM") as psp,
            tc.tile_pool(name="ps_small", bufs=2, space="PSUM") as pssp,
        ):
            # xg spill: per step a [128(j-within-tile), jb=4g+u, 32(b)] block
            d_xg = dramp.tile([T, 128, NJ, B], BF16, tag="xg_spill")
            d_ccin = dramp.tile([B, H], F32, tag="cc_in")
            d_ccout = dramp.tile([B, H], F32, tag="cc_out")

            # ---- persistent constants ----
            t_wih = constp.tile([128, 4, 4 * H], BF16, tag="wih")
            t_whh = constp.tile([128, 4, 4 * H], BF16, tag="whh")
            t_xT = constp.tile([128, 4, T * B], BF16, tag="xT")
            for k in range(4):
                sl = slice(128 * k, 128 * (k + 1))
                nc.sync.dma_start(out=t_wih[:, k, :], in_=d_wihT[sl, :])
                nc.sync.dma_start(out=t_whh[:, k, :], in_=d_whhT[sl, :])
                nc.sync.dma_start(out=t_xT[:, k, :], in_=d_xT[sl, :])
            t_biasB = constp.tile([128, NJ], F32, tag="biasB")
            t_biasrow = constp.tile([1, 4 * H], F32, tag="biasrow")
            t_gb = constp.tile([128, 8], F32, tag="gb")
            t_eye = constp.tile([32, 32], F32, tag="eye")
            nc.sync.dma_start(out=t_biasB[:], in_=d_biasB[:])
            nc.sync.dma_start(out=t_biasrow[:], in_=d_biasrow[:])
            nc.sync.dma_start(out=t_gb[:], in_=d_gb[:])
            nc.sync.dma_start(out=t_eye[:], in_=d_eye[:])
            t_ones = constp.tile([1, 128], F32, tag="ones")
            nc.vector.memset(t_ones[:], 1.0)
            t_ones32 = constp.tile([1, 32], F32, tag="ones32")
            nc.vector.memset(t_ones32[:], 1.0)
            t_eps = constp.tile([128, 1], F32, tag="eps")
            nc.vector.memset(t_eps[:], BN_EPS)

            # persistent state: h transposed, c transposed ([128, u, 32])
            t_h2t = constp.tile([128, 4, NSLOT * B], BF16, tag="h2t")
            nc.vector.memset(t_h2t[:, :, 52 * B : 53 * B], 0.0)  # pad slot
            t_cB = constp.tile([128, 4, B], F32, tag="cB")

            # ---- phase A: init matmul (DMA-bound) ----
            _sc = nc.enter_named_scope("init_mm", False)[0]
            ps_x0 = psp.tile([B, H], F32, tag="ps", name="ps_x0")
            enc_view = d_encT.ap().rearrange("(n p) b -> n p b", p=128)
            for c in range(KT_INIT // ENC_CHUNK):
                t_enc = encp.tile([128, ENC_CHUNK, B], BF16, tag="enc")
                nc.sync.dma_start(
                    out=t_enc[:],
                    in_=enc_view[c * ENC_CHUNK : (c + 1) * ENC_CHUNK, :, :].rearrange(
                        "n p b -> p n b"
                    ),
                )
                t_w = winitp.tile([128, ENC_CHUNK, H], BF16, tag="winit")
                nc.sync.dma_start(
                    out=t_w[:],
                    in_=d_initwT[
                        c * ENC_CHUNK * 128 : (c + 1) * ENC_CHUNK * 128, :
                    ].rearrange("(n p) h -> p n h", p=128),
                )
                for j in range(ENC_CHUNK):
                    k = c * ENC_CHUNK + j
                    nc.tensor.matmul(
                        ps_x0[:],
                        lhsT=t_enc[:, j, :],
                        rhs=t_w[:, j, :],
                        start=(k == 0),
                        stop=(k == KT_INIT - 1),
                    )
            nc.leave_named_scope("init_mm", _sc, False)

            # ---- phase A': x-side gates, computed transposed, spilled to HBM ----
            # xgT[j, (t,b)] = sum_e W_ih'[j, e] x[(t,b), e] + bias'[j]
            _sc = nc.enter_named_scope("xgates", False)[0]
            for jb in range(NJ):
                jsl = slice(128 * jb, 128 * (jb + 1))
                for a, bnd in XG_CHUNKS:
                    pg = psp.tile([128, 512], F32, tag="ps", name=f"xg{jb}_{a}")
                    for k in range(4):
                        nc.tensor.matmul(
                            pg[:, : bnd - a],
                            lhsT=t_wih[:, k, jsl],
                            rhs=t_xT[:, k, a:bnd],
                            start=(k == 0),
                            stop=(k == 3),
                        )
                    t_stage = xgwinp.tile([128, 512], BF16, tag="xgstage")
                    nc.vector.tensor_scalar_add(
                        t_stage[:, : bnd - a], pg[:, : bnd - a], t_biasB[:, jb : jb + 1]
                    )
                    nsteps = (bnd - a) // B
                    t0s = a // B
                    nc.sync.dma_start(
                        out=d_xg[t0s : t0s + nsteps, :, jb, :].rearrange("t p b -> p t b"),
                        in_=t_stage[:, : bnd - a].rearrange("p (t b) -> p t b", b=B),
                    )
            nc.leave_named_scope("xgates", _sc, False)

            # ---- phase B: AllReduce partial x0, BN ----
            _sc = nc.enter_named_scope("ar_bn", False)[0]
            t_x0 = workbnp.tile([B, H], F32, tag="x0")
            nc.vector.tensor_copy(t_x0[:], ps_x0[:])
            nc.sync.dma_start(out=d_ccin[:], in_=t_x0[:])
            nc.gpsimd.collective_compute(
                "AllReduce",
                mybir.AluOpType.add,
                ins=[d_ccin.opt()],
                outs=[d_ccout.opt()],
                replica_groups=[list(range(NCORES))],
            )
            t_x0r = workbnp.tile([B, H], F32, tag="x0r")
            nc.sync.dma_start(out=t_x0r[:], in_=d_ccout[:])

            # transpose x0 -> [128, 4, 32]
            ps_tp0 = pssp.tile([128, 128], F32, tag="ps_small", name="tp0")
            for u in range(4):
                nc.tensor.transpose(
                    ps_tp0[:, 32 * u : 32 * (u + 1)],
                    t_x0r[:, 128 * u : 128 * (u + 1)],
                    t_eye[:],
                )
            t_x0T = workbnp.tile([128, 4, B], F32, tag="x0T")
            nc.vector.tensor_copy(t_x0T[:], ps_tp0[:].rearrange("p (u b) -> p u b", b=B))

            # BatchNorm in transposed layout (stats along free dim = batch)
            t_xn = workbnp.tile([128, 4, B], BF16, tag="xn")
            for u in range(4):
                t_stats = workbnp.tile([128, 6], F32, tag="bnstats")
                nc.vector.bn_stats(out=t_stats[:], in_=t_x0T[:, u, :])
                t_mv = workbnp.tile([128, 2], F32, tag="bnmv")
                nc.vector.bn_aggr(out=t_mv[:], in_=t_stats[:])
                t_sd = workbnp.tile([128, 1], F32, tag="bnsd")
                nc.scalar.activation(
                    out=t_sd[:],
                    in_=t_mv[:, 1:2],
                    func=mybir.ActivationFunctionType.Sqrt,
                    bias=t_eps[:],
                )
                t_rstd = workbnp.tile([128, 1], F32, tag="bnrstd")
                nc.vector.reciprocal(t_rstd[:], t_sd[:])
                t_scale = workbnp.tile([128, 1], F32, tag="bnscale")
                nc.vector.tensor_mul(t_scale[:], t_rstd[:], t_gb[:, 2 * u : 2 * u + 1])
                nc.vector.tensor_scalar(
                    out=t_xn[:, u, :],
                    in0=t_x0T[:, u, :],
                    scalar1=t_mv[:, 0:1],
                    scalar2=t_scale[:],
                    op0=mybir.AluOpType.subtract,
                    op1=mybir.AluOpType.mult,
                )
                nc.vector.tensor_scalar_add(
                    t_xn[:, u, :], t_xn[:, u, :], t_gb[:, 2 * u + 1 : 2 * u + 2]
                )
            nc.leave_named_scope("ar_bn", _sc, False)

            def cell_b(t, w_tile, rhs_fn, win, has_f):
                """LSTM cell, transposed-gates orientation.

                psum [128, (4g, 4u, 32b)] accumulated by 64 matmuls; win is the
                x-side+bias tile [128, (g,u), 32] bf16 (None => inject bias via
                K=1 matmuls, used for the init cell)."""
                pg = psp.tile([128, 4, 4, B], F32, tag="ps", name=f"g{t}")
                for g in range(4):
                    for u in range(4):
                        jb = 4 * g + u
                        jsl = slice(128 * jb, 128 * (jb + 1))
                        if win is None:
                            nc.tensor.matmul(
                                pg[:, g, u, :],
                                lhsT=t_biasrow[:, jsl],
                                rhs=t_ones32[:],
                                start=True,
                                stop=False,
                            )
                        for k in range(4):
                            nc.tensor.matmul(
                                pg[:, g, u, :],
                                lhsT=w_tile[:, k, jsl],
                                rhs=rhs_fn(k),
                                start=(win is not None and k == 0),
                                stop=(k == 3),
                            )
                # gsum = psum (+ win): [128, (g,u,b)] bf16
                t_gs = workp.tile([128, 4, 4, B], BF16, tag="gs")
                if win is None:
                    nc.vector.tensor_copy(t_gs[:], pg[:])
                else:
                    nc.vector.tensor_add(t_gs[:], pg[:], win[:])
                # activations: tanh on g block, sigmoid on i,f,o blocks
                t_tg = workp.tile([128, 4, B], BF16, tag="tg")
                nc.scalar.activation(t_tg[:], t_gs[:, 0], mybir.ActivationFunctionType.Tanh)
                t_sig = workp.tile([128, 3, 4, B], BF16, tag="sig")
                nc.scalar.activation(
                    t_sig[:], t_gs[:, 1:4], mybir.ActivationFunctionType.Sigmoid
                )
                # c update (cB stays f32)
                t_t0 = workp.tile([128, 4, B], BF16, tag="t0")
                nc.vector.tensor_mul(t_t0[:], t_sig[:, 0], t_tg[:])
                if has_f:
                    t_c1 = workp.tile([128, 4, B], F32, tag="c1")
                    nc.vector.tensor_mul(t_c1[:], t_sig[:, 1], t_cB[:])
                    nc.vector.tensor_add(t_cB[:], t_c1[:], t_t0[:])
                else:
                    nc.vector.tensor_copy(t_cB[:], t_t0[:])
                t_tc = workp.tile([128, 4, B], BF16, tag="tc")
                nc.scalar.activation(t_tc[:], t_cB[:], mybir.ActivationFunctionType.Tanh)
                # h2 straight into the transposed h-state
                slot = t + 1
                nc.vector.tensor_mul(
                    t_h2t[:, :, B * slot : B * (slot + 1)], t_sig[:, 2], t_tc[:]
                )

            # init cell: x = BN output (rhs), h=c=0 (skip f), bias via K=1 inject
            _sc = nc.enter_named_scope("cell0", False)[0]
            cell_b(-1, t_wih, lambda k: t_xn[:, k, :], win=None, has_f=False)
            nc.leave_named_scope("cell0", _sc, False)

            # fc constants (loaded during the preamble; fcw reuses the xT slot,
            # whose last read is in xgates)
            t_fcw = constp.tile([128, 4, VS], BF16, tag="xT")
            for k in range(4):
                nc.sync.dma_start(out=t_fcw[:, k, :], in_=d_fcwT[128 * k : 128 * (k + 1), :])
            t_fcb = constp.tile([1, VS], F32, tag="fcb")
            t_mask = constp.tile([128, NM], F32, tag="mask")
            nc.sync.dma_start(out=t_fcb[:], in_=d_fcb[:])
            nc.sync.dma_start(out=t_mask[:], in_=d_mask[:])

            def fc_tile(m):
                """vocab projection for fc row-tile m (slots 4m+1 .. 4m+4)."""
                csl = slice(B + 128 * m, B + 128 * (m + 1))  # slots 1.. (skip slot 0)
                ps_c = []
                for a, bnd in FC_CHUNKS:
                    pc = psp.tile([128, 512], F32, tag="ps", name=f"fc{m}_{a}")
                    nc.tensor.matmul(
                        pc[:, : bnd - a],
                        lhsT=t_ones[:],
                        rhs=t_fcb[:, a:bnd],
                        start=True,
                        stop=False,
                    )
                    for k in range(4):
                        nc.tensor.matmul(
                            pc[:, : bnd - a],
                            lhsT=t_h2t[:, k, csl],
                            rhs=t_fcw[:, k, a:bnd],
                            start=False,
                            stop=(k == 3),
                        )
                    ps_c.append(pc)
                t_out = bigp.tile([128, VS], F32, tag="bigstage")
                for (a, bnd), pc in zip(FC_CHUNKS, ps_c):
                    nc.vector.tensor_scalar_mul(
                        t_out[:, a:bnd], pc[:, : bnd - a], t_mask[:, m : m + 1]
                    )
                nc.sync.dma_start(out=d_preds[m, :, :], in_=t_out[:])

            # ---- phase C: recurrence, with fc tiles interleaved every 4 steps ----
            _sc = nc.enter_named_scope("recur", False)[0]
            for t in range(T):
                win = xgwinp.tile([128, 4, 4, B], BF16, tag="xgwin")
                nc.sync.dma_start(
                    out=win[:], in_=d_xg[t, :, :, :].rearrange("p j b -> p (j b)")
                )
                cell_b(
                    t,
                    t_whh,
                    lambda k, t=t: t_h2t[:, k, B * t : B * (t + 1)],
                    win=win,
                    has_f=True,
                )
                if t % 4 == 3:
                    fc_tile(t // 4)
            nc.leave_named_scope("recur", _sc, False)

            # ---- phase D: fc tail (slots 49..51 + pad) ----
            _sc = nc.enter_named_scope("fc", False)[0]
            fc_tile(NM - 1)
            nc.leave_named_scope("fc", _sc, False)

    _split_multi_waits(nc)
    return nc


_NC_CACHE = None


def _get_nc():
    global _NC_CACHE
    if _NC_CACHE is None:
        _NC_CACHE = _build()
    return _NC_CACHE


def _to_bf16(a):
    """fast float32 -> bfloat16 with round-to-nearest-even (finite inputs)."""
    import ml_dtypes

    a = np.ascontiguousarray(a, dtype=np.float32)
    u = a.reshape(-1).view(np.uint32)
    r = ((u >> np.uint32(16)) & np.uint32(1)) + np.uint32(0x7FFF)
    out = ((u + r) >> np.uint32(16)).astype(np.uint16)
    return out.view(ml_dtypes.bfloat16).reshape(a.shape)


def kernel(
    encoder_out,
    encoded_captions,
    caption_lengths,
    emb_w,
    init_w,
    init_b,
    bn_gamma,
    bn_beta,
    w_ih,
    w_hh,
    b_ih,
    b_hh,
    fc_w,
    fc_b,
    **_unused,
):
    encoder_out = np.asarray(encoder_out, dtype=np.float32)
    encoded_captions = np.asarray(encoded_captions)
    caption_lengths = np.asarray(caption_lengths)
    emb_w = np.asarray(emb_w, dtype=np.float32)
    init_w = np.asarray(init_w, dtype=np.float32)
    bn_gamma = np.asarray(bn_gamma, dtype=np.float32)
    bn_beta = np.asarray(bn_beta, dtype=np.float32)
    w_ih = np.asarray(w_ih, dtype=np.float32)
    w_hh = np.asarray(w_hh, dtype=np.float32)
    b_ih = np.asarray(b_ih, dtype=np.float32)
    b_hh = np.asarray(b_hh, dtype=np.float32)
    fc_w = np.asarray(fc_w, dtype=np.float32)
    fc_b = np.asarray(fc_b, dtype=np.float32)

    # ---- host-side prep (index ops, sorting, transposes) ----
    lengths = caption_lengths[:, 0]
    sort_ind = np.argsort(-lengths, kind="stable")
    lengths_s = lengths[sort_ind]
    caps = encoded_captions[sort_ind]
    dec_len = (lengths_s - 1).astype(lengths.dtype)
    enc = encoder_out.reshape(B, -1)[sort_ind]  # [B, ENC]

    emb = emb_w[caps[:, :T]]  # [B, T, E]
    xT = np.ascontiguousarray(emb.transpose(2, 1, 0).reshape(E, T * B))  # col = t*32+b

    wihT = np.ascontiguousarray(w_ih[GATE_PERM].T)  # [H, 4H]
    whhT = np.ascontiguousarray(w_hh[GATE_PERM].T)
    bias = (b_ih + b_hh)[GATE_PERM].astype(np.float32)
    biasB = np.ascontiguousarray(bias.reshape(NJ, 128).T)  # [128, NJ]

    gb = np.empty((128, 8), np.float32)
    for u in range(4):
        gb[:, 2 * u] = bn_gamma[128 * u : 128 * (u + 1)]
        gb[:, 2 * u + 1] = bn_beta[128 * u : 128 * (u + 1)]

    # mask[r, m]: row r=32*tau+b of fc tile m covers step t=4m+tau
    mask = np.zeros((128, NM), np.float32)
    for m in range(NM):
        for tau in range(4):
            t = 4 * m + tau
            if t < T:
                mask[32 * tau : 32 * (tau + 1), m] = (dec_len > t).astype(np.float32)

    iw16 = _to_bf16(init_w)  # [H, ENC] bf16
    initwT16 = np.ascontiguousarray(iw16.T)  # [ENC, H] bf16
    enc16 = _to_bf16(enc)
    eye = np.eye(32, dtype=np.float32)
    xT16 = _to_bf16(xT)
    wihT16 = _to_bf16(wihT)
    whhT16 = _to_bf16(whhT)

    in_maps = []
    for c in range(NCORES):
        sl = slice(c * ENC_S, (c + 1) * ENC_S)
        vsl = slice(c * VS, (c + 1) * VS)
        in_maps.append(
            {
                "encT": np.ascontiguousarray(enc16[:, sl].T),
                "initwT": initwT16[sl],
                "xT": xT16,
                "wihT": wihT16,
                "whhT": whhT16,
                "biasB": biasB,
                "biasrow": bias.reshape(1, 4 * H),
                "gb": gb,
                "fcwT": _to_bf16(np.ascontiguousarray(fc_w[vsl].T)),
                "fcb": fc_b[vsl].reshape(1, VS).astype(np.float32),
                "mask": mask,
                "eye": eye,
            }
        )

    nc = _get_nc()
    res = run_bass_kernel_spmd(nc, in_maps, core_ids=list(range(NCORES)))

    # ---- assemble ----
    predictions = np.empty((B, T, V), np.float32)
    for c, r in enumerate(res.results):
        arr = r["preds"].reshape(NM, 4, B, VS)  # [m, tau, b, v]
        part = arr.transpose(2, 0, 1, 3).reshape(B, NM * 4, VS)[:, :T]
        predictions[:, :, c * VS : (c + 1) * VS] = part

    return predictions, caps, dec_len, sort_ind.astype(np.int32)


# revision 43
# speedup vs baseline: 2.8535x; 1.3355x over previous
"""Trainium2 Bass kernel: image-captioning LSTM decoder (DecoderWithRNN).

Strategy (8 NeuronCores, SPMD, one NEFF, one collective):
  - init Linear (512 x 401408, 822MB of weights) is contraction-sharded over
    encoder_dim: each core streams 1/8 of init_w^T (bf16) from HBM and
    accumulates a partial x0 [32,512]; one small AllReduce produces full x0.
  - BatchNorm (batch stats), the 51-step LSTM recurrence, and the x-side gate
    precompute are replicated on every core (the recurrence is PE-bound;
    batch replication costs nothing extra).
  - fc (vocab projection, V=10000) is sharded over V: each core emits
    preds[:, :, slice of 1250] and the host concatenates.
  - sorting / embedding lookup / weight transposes are host-side input prep.

Recurrence layout ("orientation B"): gates are computed TRANSPOSED —
  gatesT[j, b] = sum_k W'[j, k] h[k, b], with W' row-permuted to [g,i,f,o].
  j lives on partitions (16 tiles of 128), batch on the free dim. All 64
  accumulating matmuls of a step target ONE psum bank [128, (4g,4u,32b)].
  The elementwise chain then runs at full 128-partition occupancy and h2 is
  produced directly in the transposed h-state layout H2T [128, u, slot*32+b]
  (no transposes on the hot path). The x-side contributions (x_t @ W_ih^T +
  biases) for all steps are precomputed in one batched matmul, spilled to HBM
  transposed, and streamed back one [128, 512] bf16 window per step.
"""

import numpy as np

import concourse.bass as bass
import concourse.tile as tile
from concourse import mybir
from concourse.bass_utils import run_bass_kernel_spmd

B, L, V, H, E = 32, 52, 10000, 512, 512
ENC = 14 * 14 * 2048
NCORES = 8
T = L - 1  # 51 decode steps
NSLOT = 53  # h-state slots: 0 = init-cell h, 1..51 = step h2, 52 = zero pad
ENC_S = ENC // NCORES  # 50176
KT_INIT = ENC_S // 128  # 392 contraction tiles for the init matmul
ENC_CHUNK = 8  # enc^T k-tiles per DMA
VS = V // NCORES  # 1250 vocab columns per core
NM = (NSLOT * B) // 128  # 13 fc row-tiles (52 slots + pad, 32 batch)
NJ = 16  # 128-wide j tiles of the 2048 gate dim
BN_EPS = 1e-5
F32 = mybir.dt.float32
BF16 = mybir.dt.bfloat16

# gate order on device: [g, i, f, o]
GATE_PERM = np.concatenate(
    [np.arange(2 * H, 3 * H), np.arange(0, H), np.arange(H, 2 * H), np.arange(3 * H, 4 * H)]
)

FC_CHUNKS = [(0, 512), (512, 1024), (1024, VS)]
XG_CHUNKS = [(0, 512), (512, 1024), (1024, 1536), (1536, T * B)]


def _split_multi_waits(nc, max_waits=1):
    """walrus in this container allows a single sync-wait per instruction;
    split Tile's multi-wait tail drain into preceding single-wait NOPs."""
    n = 0
    for bb in nc.main_func.blocks:
        insts = bb.instructions
        i = 0
        while i < len(insts):
            inst = insts[i]
            si = getattr(inst, "sync_info", None)
            if si is not None and si.on_wait and len(si.on_wait) > max_waits:
                extra = si.on_wait[: -max_waits]
                si.on_wait = si.on_wait[-max_waits:]
                for k, w in enumerate(extra):
                    nop = mybir.InstNoOp(
                        name=f"{inst.name}-wsplit{k}",
                        sync_info=mybir.SyncInfo(on_wait=[w], on_update=[]),
                        engine=inst.engine,
                        bass_nofuse=True,
                    )
                    insts.insert(i, nop)
                    i += 1
                n += len(extra)
            i += 1
    return n


def _build():
    nc = bass.Bass("TRN2", target_bir_lowering=False, debug=False, num_devices=NCORES)

    # ---- I/O ----
    # enc/init_w streams are host-packed into partition-tiled layout
    # [128, (ktile, cols)] so every DMA is fully contiguous.
    d_encT = nc.dram_tensor("encT", [128, KT_INIT * B], BF16, kind="ExternalInput")
    d_initwT = nc.dram_tensor("initwT", [128, KT_INIT * H], BF16, kind="ExternalInput")
    d_xT = nc.dram_tensor("xT", [H, T * B], BF16, kind="ExternalInput")
    d_wihT = nc.dram_tensor("wihT", [H, 4 * H], BF16, kind="ExternalInput")
    d_whhT = nc.dram_tensor("whhT", [H, 4 * H], BF16, kind="ExternalInput")
    d_biasB = nc.dram_tensor("biasB", [128, NJ], F32, kind="ExternalInput")
    d_biasrow = nc.dram_tensor("biasrow", [1, 4 * H], F32, kind="ExternalInput")
    d_gb = nc.dram_tensor("gb", [128, 8], F32, kind="ExternalInput")
    d_fcwT = nc.dram_tensor("fcwT", [H, VS], BF16, kind="ExternalInput")
    d_fcb = nc.dram_tensor("fcb", [1, VS], F32, kind="ExternalInput")
    d_mask = nc.dram_tensor("mask", [128, NM], F32, kind="ExternalInput")
    d_eye = nc.dram_tensor("eye", [32, 32], F32, kind="ExternalInput")
    d_preds = nc.dram_tensor("preds", [NM, 128, VS], F32, kind="ExternalOutput")

    with tile.TileContext(nc) as tc:
        with (
            tc.tile_pool(name="const", bufs=1) as constp,
            tc.tile_pool(name="encs", bufs=2) as encp,
            tc.tile_pool(name="winit", bufs=3) as winitp,
            tc.tile_pool(name="bigstage", bufs=2) as bigp,
            tc.tile_pool(name="xgwin", bufs=4) as xgwinp,
            tc.tile_pool(name="work", bufs=2) as workp,
            tc.tile_pool(name="workbn", bufs=1) as workbnp,
            tc.tile_pool(name="dram", bufs=1, space="DRAM") as dramp,
            tc.tile_pool(name="ps", bufs=8, space="PSUM") as psp,
        ):
            # xg spill: [jb, 128(j-within-tile), (t,b)] — contiguous writes;
            # read back in 4-step groups as [128, jb, 4*32]
            d_xg = dramp.tile([NJ, 128, T * B], BF16, tag="xg_spill")
            d_ccin = dramp.tile([B, H], F32, tag="cc_in")
            d_ccout = dramp.tile([B, H], F32, tag="cc_out")

            # ---- persistent constants ----
            t_wih = constp.tile([128, 4, 4 * H], BF16, tag="wih")
            t_whh = constp.tile([128, 4, 4 * H], BF16, tag="whh")
            t_xT = constp.tile([128, 4, T * B], BF16, tag="xT")
            for k in range(4):
                sl = slice(128 * k, 128 * (k + 1))
                nc.sync.dma_start(out=t_wih[:, k, :], in_=d_wihT[sl, :])
                nc.sync.dma_start(out=t_whh[:, k, :], in_=d_whhT[sl, :])
                nc.sync.dma_start(out=t_xT[:, k, :], in_=d_xT[sl, :])
            t_biasB = constp.tile([128, NJ], F32, tag="biasB")
            t_biasrow = constp.tile([1, 4 * H], F32, tag="biasrow")
            t_gb = constp.tile([128, 8], F32, tag="gb")
            t_eye = constp.tile([32, 32], F32, tag="eye")
            nc.sync.dma_start(out=t_biasB[:], in_=d_biasB[:])
            nc.sync.dma_start(out=t_biasrow[:], in_=d_biasrow[:])
            nc.sync.dma_start(out=t_gb[:], in_=d_gb[:])
            nc.sync.dma_start(out=t_eye[:], in_=d_eye[:])
            t_ones = constp.tile([1, 128], F32, tag="ones")
            nc.vector.memset(t_ones[:], 1.0)
            t_ones32 = constp.tile([1, 32], F32, tag="ones32")
            nc.vector.memset(t_ones32[:], 1.0)
            t_eps = constp.tile([128, 1], F32, tag="eps")
            nc.vector.memset(t_eps[:], BN_EPS)

            # persistent state: h transposed, c transposed ([128, u, 32])
            t_h2t = constp.tile([128, 4, NSLOT * B], BF16, tag="h2t")
            nc.vector.memset(t_h2t[:, :, 52 * B : 53 * B], 0.0)  # pad slot
            t_cB = constp.tile([128, 4, B], F32, tag="cB")

            # ---- phase A: init matmul (DMA-bound) ----
            _sc = nc.enter_named_scope("init_mm", False)[0]
            ps_x0 = psp.tile([B, H], F32, tag="ps", name="ps_x0")
            for c in range(KT_INIT // ENC_CHUNK):
                t_enc = encp.tile([128, ENC_CHUNK, B], BF16, tag="enc")
                nc.sync.dma_start(
                    out=t_enc[:],
                    in_=d_encT[:, c * ENC_CHUNK * B : (c + 1) * ENC_CHUNK * B],
                )
                t_w = winitp.tile([128, ENC_CHUNK, H], BF16, tag="winit")
                nc.sync.dma_start(
                    out=t_w[:],
                    in_=d_initwT[:, c * ENC_CHUNK * H : (c + 1) * ENC_CHUNK * H],
                )
                for j in range(ENC_CHUNK):
                    k = c * ENC_CHUNK + j
                    nc.tensor.matmul(
                        ps_x0[:],
                        lhsT=t_enc[:, j, :],
                        rhs=t_w[:, j, :],
                        start=(k == 0),
                        stop=(k == KT_INIT - 1),
                    )
            nc.leave_named_scope("init_mm", _sc, False)

            # ---- phase A': x-side gates, computed transposed, spilled to HBM ----
            # xgT[j, (t,b)] = sum_e W_ih'[j, e] x[(t,b), e] + bias'[j]
            _sc = nc.enter_named_scope("xgates", False)[0]
            for jb in range(NJ):
                jsl = slice(128 * jb, 128 * (jb + 1))
                for a, bnd in XG_CHUNKS:
                    pg = psp.tile([128, 512], F32, tag="ps", name=f"xg{jb}_{a}")
                    for k in range(4):
                        nc.tensor.matmul(
                            pg[:, : bnd - a],
                            lhsT=t_wih[:, k, jsl],
                            rhs=t_xT[:, k, a:bnd],
                            start=(k == 0),
                            stop=(k == 3),
                        )
                    t_stage = xgwinp.tile([128, 512], BF16, tag="xgstage")
                    nc.vector.tensor_scalar_add(
                        t_stage[:, : bnd - a], pg[:, : bnd - a], t_biasB[:, jb : jb + 1]
                    )
                    nc.sync.dma_start(
                        out=d_xg[jb, :, a:bnd], in_=t_stage[:, : bnd - a]
                    )
            nc.leave_named_scope("xgates", _sc, False)

            # ---- phase B: AllReduce partial x0, BN ----
            _sc = nc.enter_named_scope("ar_bn", False)[0]
            t_x0 = workbnp.tile([B, H], F32, tag="x0")
            nc.vector.tensor_copy(t_x0[:], ps_x0[:])
            nc.sync.dma_start(out=d_ccin[:], in_=t_x0[:])
            nc.gpsimd.collective_compute(
                "AllReduce",
                mybir.AluOpType.add,
                ins=[d_ccin.opt()],
                outs=[d_ccout.opt()],
                replica_groups=[list(range(NCORES))],
            )
            t_x0r = workbnp.tile([B, H], F32, tag="x0r")
            nc.sync.dma_start(out=t_x0r[:], in_=d_ccout[:])

            # transpose x0 -> [128, 4, 32]
            ps_tp0 = psp.tile([128, 128], F32, tag="ps", name="tp0")
            for u in range(4):
                nc.tensor.transpose(
                    ps_tp0[:, 32 * u : 32 * (u + 1)],
                    t_x0r[:, 128 * u : 128 * (u + 1)],
                    t_eye[:],
                )
            t_x0T = workbnp.tile([128, 4, B], F32, tag="x0T")
            nc.vector.tensor_copy(t_x0T[:], ps_tp0[:].rearrange("p (u b) -> p u b", b=B))

            # BatchNorm in transposed layout (stats along free dim = batch)
            t_xn = workbnp.tile([128, 4, B], BF16, tag="xn")
            for u in range(4):
                t_stats = workbnp.tile([128, 6], F32, tag="bnstats")
                nc.vector.bn_stats(out=t_stats[:], in_=t_x0T[:, u, :])
                t_mv = workbnp.tile([128, 2], F32, tag="bnmv")
                nc.vector.bn_aggr(out=t_mv[:], in_=t_stats[:])
                t_sd = workbnp.tile([128, 1], F32, tag="bnsd")
                nc.scalar.activation(
                    out=t_sd[:],
                    in_=t_mv[:, 1:2],
                    func=mybir.ActivationFunctionType.Sqrt,
                    bias=t_eps[:],
                )
                t_rstd = workbnp.tile([128, 1], F32, tag="bnrstd")
                nc.vector.reciprocal(t_rstd[:], t_sd[:])
                t_scale = workbnp.tile([128, 1], F32, tag="bnscale")
                nc.vector.tensor_mul(t_scale[:], t_rstd[:], t_gb[:, 2 * u : 2 * u + 1])
                nc.vector.tensor_scalar(
                    out=t_xn[:, u, :],
                    in0=t_x0T[:, u, :],
                    scalar1=t_mv[:, 0:1],
                    scalar2=t_scale[:],
                    op0=mybir.AluOpType.subtract,
                    op1=mybir.AluOpType.mult,
                )
                nc.vector.tensor_scalar_add(
                    t_xn[:, u, :], t_xn[:, u, :], t_gb[:, 2 * u + 1 : 2 * u + 2]
                )
            nc.leave_named_scope("ar_bn", _sc, False)

            def cell_b(t, w_tile, rhs_fn, win, has_f):
                """LSTM cell, transposed-gates orientation, gate-pipelined.

                Per gate g: one psum bank [128, (4u, 32b)] accumulated by 16
                matmuls, then gsum-add + activation immediately (overlapping
                later gates' matmuls). win is the x-side+bias window slice
                [128, (g,u), 32] bf16 (None => inject bias via K=1 matmuls)."""
                t_gs = workp.tile([128, 4, 4, B], BF16, tag="gs")
                t_tg = workp.tile([128, 4, B], BF16, tag="tg")
                t_sig = workp.tile([128, 3, 4, B], BF16, tag="sig")
                t_t0 = workp.tile([128, 4, B], BF16, tag="t0")
                t_tc = workp.tile([128, 4, B], BF16, tag="tc")
                for g in range(4):
                    pg = psp.tile([128, 4, B], F32, tag="ps", name=f"g{t}_{g}")
                    for u in range(4):
                        jb = 4 * g + u
                        jsl = slice(128 * jb, 128 * (jb + 1))
                        if win is None:
                            nc.tensor.matmul(
                                pg[:, u, :],
                                lhsT=t_biasrow[:, jsl],
                                rhs=t_ones32[:],
                                start=True,
                                stop=False,
                            )
                        for k in range(4):
                            nc.tensor.matmul(
                                pg[:, u, :],
                                lhsT=w_tile[:, k, jsl],
                                rhs=rhs_fn(k),
                                start=(win is not None and k == 0),
                                stop=(k == 3),
                            )
                    # gsum for this gate, then its activation right away
                    if win is None:
                        nc.vector.tensor_copy(t_gs[:, g], pg[:])
                    else:
                        nc.vector.tensor_add(t_gs[:, g], pg[:], win(g))
                    if g == 0:
                        nc.scalar.activation(
                            t_tg[:], t_gs[:, 0], mybir.ActivationFunctionType.Tanh
                        )
                    else:
                        nc.scalar.activation(
                            t_sig[:, g - 1],
                            t_gs[:, g],
                            mybir.ActivationFunctionType.Sigmoid,
                        )
                    # c-chain interleaves with later gates' matmuls
                    if g == 1:
                        nc.vector.tensor_mul(t_t0[:], t_sig[:, 0], t_tg[:])
                    elif g == 2:
                        if has_f:
                            t_c1 = workp.tile([128, 4, B], F32, tag="c1")
                            nc.vector.tensor_mul(t_c1[:], t_sig[:, 1], t_cB[:])
                            nc.vector.tensor_add(t_cB[:], t_c1[:], t_t0[:])
                        else:
                            nc.vector.tensor_copy(t_cB[:], t_t0[:])
                        nc.scalar.activation(
                            t_tc[:], t_cB[:], mybir.ActivationFunctionType.Tanh
                        )
                # h2 straight into the transposed h-state
                slot = t + 1
                nc.vector.tensor_mul(
                    t_h2t[:, :, B * slot : B * (slot + 1)], t_sig[:, 2], t_tc[:]
                )

            # init cell: x = BN output (rhs), h=c=0 (skip f), bias via K=1 inject
            _sc = nc.enter_named_scope("cell0", False)[0]
            cell_b(-1, t_wih, lambda k: t_xn[:, k, :], win=None, has_f=False)
            nc.leave_named_scope("cell0", _sc, False)
            del t_xn

            # fc constants (loaded during the preamble; fcw reuses the xT slot,
            # whose last read is in xgates)
            t_fcw = constp.tile([128, 4, VS], BF16, tag="xT")
            for k in range(4):
                nc.sync.dma_start(out=t_fcw[:, k, :], in_=d_fcwT[128 * k : 128 * (k + 1), :])
            t_fcb = constp.tile([1, VS], F32, tag="fcb")
            t_mask = constp.tile([128, NM], F32, tag="mask")
            nc.sync.dma_start(out=t_fcb[:], in_=d_fcb[:])
            nc.sync.dma_start(out=t_mask[:], in_=d_mask[:])

            def fc_tile(m):
                """vocab projection for fc row-tile m (slots 4m+1 .. 4m+4)."""
                csl = slice(B + 128 * m, B + 128 * (m + 1))  # slots 1.. (skip slot 0)
                ps_c = []
                for a, bnd in FC_CHUNKS:
                    pc = psp.tile([128, 512], F32, tag="ps", name=f"fc{m}_{a}")
                    nc.tensor.matmul(
                        pc[:, : bnd - a],
                        lhsT=t_ones[:],
                        rhs=t_fcb[:, a:bnd],
                        start=True,
                        stop=False,
                    )
                    for k in range(4):
                        nc.tensor.matmul(
                            pc[:, : bnd - a],
                            lhsT=t_h2t[:, k, csl],
                            rhs=t_fcw[:, k, a:bnd],
                            start=False,
                            stop=(k == 3),
                        )
                    ps_c.append(pc)
                t_out = bigp.tile([128, VS], F32, tag="bigstage")
                for (a, bnd), pc in zip(FC_CHUNKS, ps_c):
                    nc.vector.tensor_scalar_mul(
                        t_out[:, a:bnd], pc[:, : bnd - a], t_mask[:, m : m + 1]
                    )
                nc.sync.dma_start(out=d_preds[m, :, :], in_=t_out[:])

            # ---- phase C: recurrence, with fc tiles interleaved every 4 steps ----
            _sc = nc.enter_named_scope("recur", False)[0]
            win4 = None
            for t in range(T):
                if t % 4 == 0:
                    # window for steps t..t+3: [128, jb, tau*32+b]
                    ncols = min(4 * B, T * B - t * B)
                    win4 = xgwinp.tile([128, NJ, 4 * B], BF16, tag="xgwin")
                    nc.sync.dma_start(
                        out=win4[:, :, :ncols],
                        in_=d_xg[:, :, B * t : B * t + ncols].rearrange("j p c -> p j c"),
                    )
                tau = t % 4
                w4 = win4
                cell_b(
                    t,
                    t_whh,
                    lambda k, t=t: t_h2t[:, k, B * t : B * (t + 1)],
                    win=lambda g, w4=w4, tau=tau: w4[
                        :, 4 * g : 4 * g + 4, B * tau : B * (tau + 1)
                    ],
                    has_f=True,
                )
                if t % 4 == 3:
                    fc_tile(t // 4)
            nc.leave_named_scope("recur", _sc, False)

            # ---- phase D: fc tail (slots 49..51 + pad) ----
            _sc = nc.enter_named_scope("fc", False)[0]
            fc_tile(NM - 1)
            nc.leave_named_scope("fc", _sc, False)

    _split_multi_waits(nc)
    return nc


_NC_CACHE = None


def _get_nc():
    global _NC_CACHE
    if _NC_CACHE is None:
        _NC_CACHE = _build()
    return _NC_CACHE


def _to_bf16(a):
    """fast float32 -> bfloat16 with round-to-nearest-even (finite inputs)."""
    import ml_dtypes

    a = np.ascontiguousarray(a, dtype=np.float32)
    u = a.reshape(-1).view(np.uint32)
    r = ((u >> np.uint32(16)) & np.uint32(1)) + np.uint32(0x7FFF)
    out = ((u + r) >> np.uint32(16)).astype(np.uint16)
    return out.view(ml_dtypes.bfloat16).reshape(a.shape)


def kernel(
    encoder_out,
    encoded_captions,
    caption_lengths,
    emb_w,
    init_w,
    init_b,
    bn_gamma,
    bn_beta,
    w_ih,
    w_hh,
    b_ih,
    b_hh,
    fc_w,
    fc_b,
    **_unused,
):
    encoder_out = np.asarray(encoder_out, dtype=np.float32)
    encoded_captions = np.asarray(encoded_captions)
    caption_lengths = np.asarray(caption_lengths)
    emb_w = np.asarray(emb_w, dtype=np.float32)
    init_w = np.asarray(init_w, dtype=np.float32)
    bn_gamma = np.asarray(bn_gamma, dtype=np.float32)
    bn_beta = np.asarray(bn_beta, dtype=np.float32)
    w_ih = np.asarray(w_ih, dtype=np.float32)
    w_hh = np.asarray(w_hh, dtype=np.float32)
    b_ih = np.asarray(b_ih, dtype=np.float32)
    b_hh = np.asarray(b_hh, dtype=np.float32)
    fc_w = np.asarray(fc_w, dtype=np.float32)
    fc_b = np.asarray(fc_b, dtype=np.float32)

    # ---- host-side prep (index ops, sorting, transposes) ----
    lengths = caption_lengths[:, 0]
    sort_ind = np.argsort(-lengths, kind="stable")
    lengths_s = lengths[sort_ind]
    caps = encoded_captions[sort_ind]
    dec_len = (lengths_s - 1).astype(lengths.dtype)
    enc = encoder_out.reshape(B, -1)[sort_ind]  # [B, ENC]

    emb = emb_w[caps[:, :T]]  # [B, T, E]
    xT = np.ascontiguousarray(emb.transpose(2, 1, 0).reshape(E, T * B))  # col = t*32+b

    wihT = np.ascontiguousarray(w_ih[GATE_PERM].T)  # [H, 4H]
    whhT = np.ascontiguousarray(w_hh[GATE_PERM].T)
    bias = (b_ih + b_hh)[GATE_PERM].astype(np.float32)
    biasB = np.ascontiguousarray(bias.reshape(NJ, 128).T)  # [128, NJ]

    gb = np.empty((128, 8), np.float32)
    for u in range(4):
        gb[:, 2 * u] = bn_gamma[128 * u : 128 * (u + 1)]
        gb[:, 2 * u + 1] = bn_beta[128 * u : 128 * (u + 1)]

    # mask[r, m]: row r=32*tau+b of fc tile m covers step t=4m+tau
    mask = np.zeros((128, NM), np.float32)
    for m in range(NM):
        for tau in range(4):
            t = 4 * m + tau
            if t < T:
                mask[32 * tau : 32 * (tau + 1), m] = (dec_len > t).astype(np.float32)

    iw16 = _to_bf16(init_w)  # [H, ENC] bf16
    initwT16 = np.ascontiguousarray(iw16.T)  # [ENC, H] bf16
    enc16 = _to_bf16(enc)
    eye = np.eye(32, dtype=np.float32)
    xT16 = _to_bf16(xT)
    wihT16 = _to_bf16(wihT)
    whhT16 = _to_bf16(whhT)

    def _pack(a):
        """[ENC_S, C] -> partition-tiled [128, ktile*C] (contiguous DMA layout)."""
        n, cdim = a.shape[0] // 128, a.shape[1]
        return np.ascontiguousarray(
            a.reshape(n, 128, cdim).transpose(1, 0, 2).reshape(128, n * cdim)
        )

    in_maps = []
    for c in range(NCORES):
        sl = slice(c * ENC_S, (c + 1) * ENC_S)
        vsl = slice(c * VS, (c + 1) * VS)
        in_maps.append(
            {
                "encT": _pack(np.ascontiguousarray(enc16[:, sl].T)),
                "initwT": _pack(initwT16[sl]),
                "xT": xT16,
                "wihT": wihT16,
                "whhT": whhT16,
                "biasB": biasB,
                "biasrow": bias.reshape(1, 4 * H),
                "gb": gb,
                "fcwT": _to_bf16(np.ascontiguousarray(fc_w[vsl].T)),
                "fcb": fc_b[vsl].reshape(1, VS).astype(np.float32),
                "mask": mask,
                "eye": eye,
            }
        )

    nc = _get_nc()
    res = run_bass_kernel_spmd(nc, in_maps, core_ids=list(range(NCORES)))

    # ---- assemble ----
    predictions = np.empty((B, T, V), np.float32)
    for c, r in enumerate(res.results):
        arr = r["preds"].reshape(NM, 4, B, VS)  # [m, tau, b, v]
        part = arr.transpose(2, 0, 1, 3).reshape(B, NM * 4, VS)[:, :T]
        predictions[:, :, c * VS : (c + 1) * VS] = part

    return predictions, caps, dec_len, sort_ind.astype(np.int32)
